# revision 18
# baseline (speedup 1.0000x reference)
"""Trainium2 Bass kernel for nn_EncoderBlock_47528108097844.

Sharding: core c handles batch b0=c//2 and heads [4*(c%2), 4*(c%2)+4) for the
attention, which (because of the reference's .view-reinterpret scramble) exactly
produces output tokens Y[:, c//2, n' in [512*(c%2), 512*(c%2)+512), :].  The
post-attention MLP is token-parallel on those 2048 tokens per core.  A host-side
query permutation sigma makes the scramble coalesce so the PV matmuls write the
final token layout directly.
"""
import numpy as np

B, A, N, H, HEADS, FFN = 4, 4, 1024, 256, 8, 512
D = 32
EPS = 1e-5
NCORES = 8

_CACHE = {}


def _sigma():
    c2 = np.arange(1024)
    hh = c2 // 128
    r = c2 % 128
    bp = r // 32
    j = r % 32
    return (j // 4) * 128 + (j % 4) * 32 + hh * 4 + bp


def _ln_stats(nc, mybir, msp, s, q, m, r, tt):
    """m = s/256 ; var = q/256 + eps - m*m ; r = exp(-0.5*ln(var))."""
    OP = mybir.AluOpType
    AF = mybir.ActivationFunctionType
    f32 = mybir.dt.float32
    sl = slice(tt * 4, (tt + 1) * 4)
    nc.vector.tensor_scalar(m[:, sl], s[:, sl], 1.0 / 256.0, None, OP.mult)
    mm = msp.tile([128, 4], f32, tag="mm")
    nc.vector.tensor_mul(mm[:], m[:, sl], m[:, sl])
    var = msp.tile([128, 4], f32, tag="var")
    nc.vector.tensor_scalar(var[:], q[:, sl], 1.0 / 256.0, EPS, OP.mult, OP.add)
    nc.vector.tensor_sub(var[:], var[:], mm[:])
    lnv = msp.tile([128, 4], f32, tag="lnv")
    nc.scalar.activation(lnv[:], var[:], AF.Ln)
    nc.scalar.activation(r[:, sl], lnv[:], AF.Exp, scale=-0.5)


def _build_program():
    import os
    PH = int(os.environ.get("KPHASES", "3"))
    SUB = int(os.environ.get("KSUB", "9"))
    import concourse.bacc as bacc
    import concourse.mybir as mybir
    import concourse.tile as tile
    from contextlib import ExitStack

    f32 = mybir.dt.float32
    bf16 = mybir.dt.bfloat16
    AF = mybir.ActivationFunctionType
    OP = mybir.AluOpType

    nc = bacc.Bacc(
        "TRN2",
        target_bir_lowering=False,
        debug=False,
        enable_asserts=False,
        num_devices=NCORES,
    )

    # ---- DRAM I/O -------------------------------------------------------
    xt_d = nc.dram_tensor("xt", [128, 2, 4096], bf16, kind="ExternalInput").ap()
    xmlp_d = nc.dram_tensor("xmlp", [2048, 256], f32, kind="ExternalInput").ap()
    mask_d = nc.dram_tensor("maskp", [128, 8, 1024], bf16, kind="ExternalInput").ap()
    wq_d = nc.dram_tensor("wq", [128, 2, 128], bf16, kind="ExternalInput").ap()
    wk_d = nc.dram_tensor("wk", [128, 2, 128], bf16, kind="ExternalInput").ap()
    wv_d = nc.dram_tensor("wv", [128, 2, 128], bf16, kind="ExternalInput").ap()
    wo_d = nc.dram_tensor("wo", [128, 2, 256], bf16, kind="ExternalInput").ap()
    w1_d = nc.dram_tensor("w1", [128, 2, 512], bf16, kind="ExternalInput").ap()
    w2_d = nc.dram_tensor("w2", [128, 4, 256], bf16, kind="ExternalInput").ap()
    b1_d = nc.dram_tensor("b1", [1, 512], bf16, kind="ExternalInput").ap()
    b2_d = nc.dram_tensor("b2", [1, 256], bf16, kind="ExternalInput").ap()
    g1_d = nc.dram_tensor("g1r", [128, 256], f32, kind="ExternalInput").ap()
    be1_d = nc.dram_tensor("be1r", [128, 256], f32, kind="ExternalInput").ap()
    g2_d = nc.dram_tensor("g2r", [128, 256], f32, kind="ExternalInput").ap()
    be2_d = nc.dram_tensor("be2r", [128, 256], f32, kind="ExternalInput").ap()
    out_d = nc.dram_tensor("out", [2048, 256], f32, kind="ExternalOutput").ap()

    with tile.TileContext(nc) as tc, ExitStack() as ctx:
        # ---- persistent SBUF -------------------------------------------
        wp = ctx.enter_context(tc.tile_pool(name="wp", bufs=1))
        mask_t = wp.tile([128, 8, 1024], bf16, tag="mask")
        wq_t = wp.tile([128, 2, 128], bf16, tag="wq")
        wk_t = wp.tile([128, 2, 128], bf16, tag="wk")
        wv_t = wp.tile([128, 2, 128], bf16, tag="wv")
        wo_t = wp.tile([128, 2, 256], bf16, tag="wo")
        w1_t = wp.tile([128, 2, 512], bf16, tag="w1")
        w2_t = wp.tile([128, 4, 256], bf16, tag="w2")
        b1_t = wp.tile([1, 512], bf16, tag="b1")
        b2_t = wp.tile([1, 256], bf16, tag="b2")
        g1_t = wp.tile([128, 256], f32, tag="g1")
        be1_t = wp.tile([128, 256], f32, tag="be1")
        g2_t = wp.tile([128, 256], f32, tag="g2")
        be2_t = wp.tile([128, 256], f32, tag="be2")
        ones_t = wp.tile([1, 512], bf16, tag="ones")

        nc.sync.dma_start(mask_t[:], mask_d)
        nc.sync.dma_start(wq_t[:], wq_d)
        nc.sync.dma_start(wk_t[:], wk_d)
        nc.sync.dma_start(wv_t[:], wv_d)
        nc.sync.dma_start(wo_t[:], wo_d)
        nc.sync.dma_start(w1_t[:], w1_d)
        nc.sync.dma_start(w2_t[:], w2_d)
        nc.sync.dma_start(b1_t[:], b1_d)
        nc.sync.dma_start(b2_t[:], b2_d)
        nc.sync.dma_start(g1_t[:], g1_d)
        nc.sync.dma_start(be1_t[:], be1_d)
        nc.sync.dma_start(g2_t[:], g2_d)
        nc.sync.dma_start(be2_t[:], be2_d)
        nc.vector.memset(ones_t[:], 1.0)

        qkp = ctx.enter_context(tc.tile_pool(name="qkp", bufs=1))
        # [d + 32*(hl%2), hl//2, t] — matmul operand bases must be 0/32/64
        qT = qkp.tile([64, 2, 4096], bf16, tag="qT")
        kT = qkp.tile([64, 2, 4096], bf16, tag="kT")
        vtok = qkp.tile([128, 32, 4, 33], bf16, tag="vtok")
        nc.gpsimd.memset(vtok[:], 1.0)  # 33rd columns stay 1.0 (PV rowsum)

        app = ctx.enter_context(tc.tile_pool(name="app", bufs=1))
        attnT = app.tile([128, 2, 2048], bf16, tag="attnT")
        rcp_t = app.tile([128, 16, 8], f32, tag="rcp")
        xmlp_t = app.tile([128, 16, 256], f32, tag="xmlp")
        y1f = app.tile([128, 16, 256], f32, tag="y1f")
        s1 = app.tile([128, 16], f32, tag="s1")
        q1 = app.tile([128, 16], f32, tag="q1")
        m1 = app.tile([128, 16], f32, tag="m1")
        r1 = app.tile([128, 16], f32, tag="r1")
        s2 = app.tile([128, 16], f32, tag="s2")
        q2 = app.tile([128, 16], f32, tag="q2")
        m2 = app.tile([128, 16], f32, tag="m2")
        r2 = app.tile([128, 16], f32, tag="r2")

        nc.sync.dma_start(xmlp_t[:], xmlp_d.rearrange("(c p) f -> p c f", p=128))

        # ---- phase 1: QKV projections ----------------------------------
        with tc.tile_pool(name="xtp", bufs=1) as xtp, \
             tc.tile_pool(name="pqkv", bufs=2, space="PSUM") as pqkv, \
             tc.tile_pool(name="pvv", bufs=2, space="PSUM") as pvv:
            xt_t = xtp.tile([128, 2, 4096], bf16, tag="xt")
            nc.sync.dma_start(xt_t[:], xt_d)

            for proj, (wt, dst) in enumerate(((wq_t, qT), (wk_t, kT))):
                for hlp in range(2):
                    for tt in range(8):
                        acc = pqkv.tile([64, 512], f32, tag="pq")
                        for kc in range(2):
                            nc.tensor.matmul(
                                acc[:],
                                wt[:, kc, hlp * 64:(hlp + 1) * 64],
                                xt_t[:, kc, tt * 512:(tt + 1) * 512],
                                start=(kc == 0),
                                stop=(kc == 1),
                            )
                        if proj == 0:
                            nc.vector.tensor_copy(
                                dst[0:64, hlp, tt * 512:(tt + 1) * 512], acc[:])
                        else:
                            nc.scalar.copy(
                                dst[0:64, hlp, tt * 512:(tt + 1) * 512], acc[:])

            for tch in range(32):
                accv = pvv.tile([128, 128], f32, tag="pv")
                for kc in range(2):
                    nc.tensor.matmul(
                        accv[:],
                        xt_t[:, kc, tch * 128:(tch + 1) * 128],
                        wv_t[:, kc, :],
                        start=(kc == 0),
                        stop=(kc == 1),
                    )
                nc.vector.tensor_copy(
                    vtok[:, tch, :, 0:32],
                    accv[:].rearrange("p (hl d) -> p hl d", hl=4),
                )

        if PH < 2:
            dbg = app.tile([128, 256], f32, tag="dbg")
            nc.vector.tensor_copy(dbg[0:64, 0:64], qT[0:64, 0, 0:64])
            nc.vector.tensor_copy(dbg[0:64, 64:128], kT[0:64, 0, 0:64])
            nc.vector.tensor_copy(
                dbg[:, 128:256].rearrange("p (a c) -> p a c", a=4),
                vtok[:, 0, :, 0:32])
            nc.sync.dma_start(out_d[0:128, :], dbg[:])

        if PH >= 2:
            # ---- phase 2: attention groups -----------------------------
            pe = ctx.enter_context(tc.tile_pool(name="pe", bufs=2, space="PSUM"))
            pat = ctx.enter_context(tc.tile_pool(name="pat", bufs=2, space="PSUM"))
            pmlp = ctx.enter_context(tc.tile_pool(name="pmlp", bufs=2, space="PSUM"))
            pp = ctx.enter_context(tc.tile_pool(name="pp", bufs=2))
            akp = ctx.enter_context(tc.tile_pool(name="akp", bufs=2))

            for g in range(16):
                hl, a = g // 4, g % 4
                P_t = pp.tile([128, 8, 1024], bf16, tag="P")
                for mc in range(8):
                    e_t = pe.tile([128, 1024], f32, tag="e")
                    p0 = (hl % 2) * 32
                    hc = hl // 2
                    for hn in range(2):
                        nc.tensor.matmul(
                            e_t[:, hn * 512:(hn + 1) * 512],
                            kT[p0:p0 + 32, hc,
                               a * 1024 + mc * 128: a * 1024 + (mc + 1) * 128],
                            qT[p0:p0 + 32, hc,
                               a * 1024 + hn * 512: a * 1024 + (hn + 1) * 512],
                            start=True,
                            stop=True,
                        )
                    nc.scalar.activation(P_t[:, mc, :], e_t[:], AF.Exp, scale=0.125)
                    if mc == 3:
                        nc.vector.tensor_mul(P_t[:, 0:4, :], P_t[:, 0:4, :],
                                             mask_t[:, 0:4, :])
                    if mc == 7:
                        nc.gpsimd.tensor_mul(P_t[:, 4:8, :], P_t[:, 4:8, :],
                                             mask_t[:, 4:8, :])

                ap_t = pat.tile([128, 8, 33], f32, tag="at")
                for hh in range(8):
                    for mc in range(8):
                        nc.tensor.matmul(
                            ap_t[:, hh, :],
                            P_t[:, mc, hh * 128:(hh + 1) * 128],
                            vtok[:, a * 8 + mc, hl, :],
                            start=(mc == 0),
                            stop=(mc == 7),
                        )

                # normalize: att = S * (1/rowsum)
                nc.vector.reciprocal(rcp_t[:, g, :], ap_t[:, :, 32])
                atok_t = akp.tile([128, 8, 32], bf16, tag="atok")
                nc.vector.tensor_mul(atok_t[:], ap_t[:, :, 0:32],
                                     rcp_t[:, g, :].to_broadcast((128, 8, 32)))
                # transpose [t, f] -> [f, t] blocks into attnT
                for fc in range(2):
                    nc.sync.dma_start_transpose(
                        attnT[:, fc, g * 128:(g + 1) * 128],
                        atok_t[:, fc * 4:(fc + 1) * 4, :],
                    )

            if PH == 2:
                dbg = app.tile([128, 256], f32, tag="dbg")
                nc.vector.tensor_copy(dbg[:], attnT[:, 0, 0:256])
                nc.sync.dma_start(out_d[0:128, :], dbg[:])

        if PH >= 3:
            # ---- phase 3: MLP over 4 t-tiles of 512 tokens -------------
            mwp = ctx.enter_context(tc.tile_pool(name="mwp", bufs=2))
            mzp = ctx.enter_context(tc.tile_pool(name="mzp", bufs=2))
            msp = ctx.enter_context(tc.tile_pool(name="msp", bufs=2))

            for tt in range(4):
                # Wo, relu
                woT_t = mwp.tile([128, 2, 512], bf16, tag="woT")
                for f2c in range(2):
                    pm = pmlp.tile([128, 512], f32, tag="pm")
                    for kc in range(2):
                        nc.tensor.matmul(
                            pm[:],
                            wo_t[:, kc, f2c * 128:(f2c + 1) * 128],
                            attnT[:, kc, tt * 512:(tt + 1) * 512],
                            start=(kc == 0),
                            stop=(kc == 1),
                        )
                    nc.vector.tensor_scalar_max(woT_t[:, f2c, :], pm[:], 0.0)
                if SUB < 2:
                    continue
                wot_t = mwp.tile([128, 4, 256], bf16, tag="wot")
                for tcl in range(4):
                    for f2c in range(2):
                        nc.sync.dma_start_transpose(
                            wot_t[:, tcl, f2c * 128:(f2c + 1) * 128],
                            woT_t[:, f2c, tcl * 128:(tcl + 1) * 128],
                        )
                if SUB < 3:
                    continue
                # LN1
                z1_t = mzp.tile([128, 4, 256], f32, tag="z1")
                zq_t = mzp.tile([128, 4, 256], bf16, tag="zq")
                for tcl in range(4):
                    tg = tt * 4 + tcl
                    nc.vector.tensor_add(
                        z1_t[:, tcl, :], wot_t[:, tcl, :], xmlp_t[:, tg, :])
                    nc.vector.tensor_reduce(
                        s1[:, tg:tg + 1], z1_t[:, tcl, :],
                        axis=mybir.AxisListType.X, op=OP.add)
                    nc.scalar.activation(
                        zq_t[:, tcl, :], z1_t[:, tcl, :], AF.Square,
                        accum_out=q1[:, tg:tg + 1])
                _ln_stats(nc, mybir, msp, s1, q1, m1, r1, tt)
                xg_t = mzp.tile([128, 4, 256], bf16, tag="xg")
                for tcl in range(4):
                    tg = tt * 4 + tcl
                    xn_t = mzp.tile([128, 256], f32, tag="xn")
                    nc.vector.tensor_scalar(
                        xn_t[:], z1_t[:, tcl, :], m1[:, tg:tg + 1], r1[:, tg:tg + 1],
                        OP.subtract, OP.mult,
                    )
                    nc.gpsimd.tensor_mul(y1f[:, tg, :], xn_t[:], g1_t[:])
                    nc.gpsimd.tensor_add(y1f[:, tg, :], y1f[:, tg, :], be1_t[:])
                    nc.gpsimd.tensor_copy(xg_t[:, tcl, :], y1f[:, tg, :])
                if SUB < 4:
                    continue
                xgT_t = mwp.tile([128, 2, 512], bf16, tag="xgT")
                for tcl in range(4):
                    for fc in range(2):
                        nc.sync.dma_start_transpose(
                            xgT_t[:, fc, tcl * 128:(tcl + 1) * 128],
                            xg_t[:, tcl, fc * 128:(fc + 1) * 128],
                        )
                # FFN1 (+b1 via rank-1 ones augmentation), relu
                F_t = mwp.tile([128, 4, 512], bf16, tag="F")
                for f2c in range(4):
                    pm = pmlp.tile([128, 512], f32, tag="pm")
                    for kc in range(2):
                        nc.tensor.matmul(
                            pm[:],
                            w1_t[:, kc, f2c * 128:(f2c + 1) * 128],
                            xgT_t[:, kc, :],
                            start=(kc == 0),
                            stop=False,
                        )
                    nc.tensor.matmul(
                        pm[:],
                        b1_t[:, f2c * 128:(f2c + 1) * 128],
                        ones_t[:, 0:512],
                        start=False,
                        stop=True,
                    )
                    nc.scalar.activation(F_t[:, f2c, :], pm[:], AF.Relu)
                if SUB < 5:
                    continue
                # FFN2 (+b2)
                f2T_t = mwp.tile([128, 2, 512], bf16, tag="f2T")
                for f2c in range(2):
                    pm = pmlp.tile([128, 512], f32, tag="pm")
                    for kc in range(4):
                        nc.tensor.matmul(
                            pm[:],
                            w2_t[:, kc, f2c * 128:(f2c + 1) * 128],
                            F_t[:, kc, :],
                            start=(kc == 0),
                            stop=False,
                        )
                    nc.tensor.matmul(
                        pm[:],
                        b2_t[:, f2c * 128:(f2c + 1) * 128],
                        ones_t[:, 0:512],
                        start=False,
                        stop=True,
                    )
                    nc.vector.tensor_copy(f2T_t[:, f2c, :], pm[:])
                f2t_t = mwp.tile([128, 4, 256], bf16, tag="f2t")
                for tcl in range(4):
                    for f2c in range(2):
                        nc.sync.dma_start_transpose(
                            f2t_t[:, tcl, f2c * 128:(f2c + 1) * 128],
                            f2T_t[:, f2c, tcl * 128:(tcl + 1) * 128],
                        )
                # LN2
                z2_t = mzp.tile([128, 4, 256], f32, tag="z2")
                zq2_t = mzp.tile([128, 4, 256], bf16, tag="zq")
                for tcl in range(4):
                    tg = tt * 4 + tcl
                    nc.vector.tensor_add(
                        z2_t[:, tcl, :], f2t_t[:, tcl, :], y1f[:, tg, :])
                    nc.vector.tensor_reduce(
                        s2[:, tg:tg + 1], z2_t[:, tcl, :],
                        axis=mybir.AxisListType.X, op=OP.add)
                    nc.scalar.activation(
                        zq2_t[:, tcl, :], z2_t[:, tcl, :], AF.Square,
                        accum_out=q2[:, tg:tg + 1])
                _ln_stats(nc, mybir, msp, s2, q2, m2, r2, tt)
                for tcl in range(4):
                    tg = tt * 4 + tcl
                    xn_t = mzp.tile([128, 256], f32, tag="xn2")
                    nc.vector.tensor_scalar(
                        xn_t[:], z2_t[:, tcl, :], m2[:, tg:tg + 1], r2[:, tg:tg + 1],
                        OP.subtract, OP.mult,
                    )
                    out_t = mzp.tile([128, 256], f32, tag="outt")
                    nc.gpsimd.tensor_mul(out_t[:], xn_t[:], g2_t[:])
                    nc.gpsimd.tensor_add(out_t[:], out_t[:], be2_t[:])
                    nc.sync.dma_start(
                        out_d[tg * 128:(tg + 1) * 128, :],
                        out_t[:],
                    )

    nc.compile()
    return nc


def _prep_inputs(inputs):
    import ml_dtypes
    bf = ml_dtypes.bfloat16

    X = np.asarray(inputs["X"], dtype=np.float32)
    Wq = np.asarray(inputs["Wq"], dtype=np.float32)
    Wk = np.asarray(inputs["Wk"], dtype=np.float32)
    Wv = np.asarray(inputs["Wv"], dtype=np.float32)
    Wo = np.asarray(inputs["Wo"], dtype=np.float32)
    w1 = np.asarray(inputs["w1"], dtype=np.float32)
    b1 = np.asarray(inputs["b1"], dtype=np.float32)
    w2 = np.asarray(inputs["w2"], dtype=np.float32)
    b2 = np.asarray(inputs["b2"], dtype=np.float32)
    g1 = np.asarray(inputs["g1"], dtype=np.float32)
    be1 = np.asarray(inputs["be1"], dtype=np.float32)
    g2 = np.asarray(inputs["g2"], dtype=np.float32)
    be2 = np.asarray(inputs["be2"], dtype=np.float32)
    adj = np.asarray(inputs["adj"])

    sig = _sigma()
    maskP = (adj[sig[None, :], sig[:, None]] > 0)  # [m', c2]
    mask_dev = np.ascontiguousarray(
        maskP.reshape(8, 128, 1024).transpose(1, 0, 2)
    ).astype(bf)

    def chunk_rows(w, kc):
        K, F = w.shape
        return np.ascontiguousarray(w.reshape(kc, 128, F).transpose(1, 0, 2))

    wo_dev = chunk_rows(Wo, 2).astype(bf)
    w1_dev = chunk_rows(w1, 2).astype(bf)
    w2_dev = chunk_rows(w2, 4).astype(bf)
    b1_dev = b1.reshape(1, 512).astype(bf)
    b2_dev = b2.reshape(1, 256).astype(bf)
    g1_dev = np.ascontiguousarray(np.broadcast_to(g1, (128, 256))).astype(np.float32)
    be1_dev = np.ascontiguousarray(np.broadcast_to(be1, (128, 256))).astype(np.float32)
    g2_dev = np.ascontiguousarray(np.broadcast_to(g2, (128, 256))).astype(np.float32)
    be2_dev = np.ascontiguousarray(np.broadcast_to(be2, (128, 256))).astype(np.float32)

    tk = np.arange(2048)
    g_t = tk // 128
    tl = tk % 128
    bp_t = tl // 32
    j_t = tl % 32
    hl_t = g_t // 4
    a_t = g_t % 4

    in_maps = []
    scatter = []
    for c in range(NCORES):
        b0, half = c // 2, c % 2
        Xp = X[b0][:, sig, :]                       # [4, 1024, 256]
        XT = Xp.reshape(4096, 256).T                # [256, 4096]
        xt_dev = np.ascontiguousarray(
            XT.reshape(2, 128, 4096).transpose(1, 0, 2)
        ).astype(bf)
        wq_dev = chunk_rows(Wq[:, half * 128:(half + 1) * 128], 2).astype(bf)
        wk_dev = chunk_rows(Wk[:, half * 128:(half + 1) * 128], 2).astype(bf)
        wv_dev = chunk_rows(Wv[:, half * 128:(half + 1) * 128], 2).astype(bf)
        np_t = (half * 4 + hl_t) * 128 + a_t * 32 + j_t
        xmlp_dev = np.ascontiguousarray(X[bp_t, b0, np_t, :]).astype(np.float32)
        in_maps.append({
            "xt": xt_dev, "xmlp": xmlp_dev, "maskp": mask_dev,
            "wq": wq_dev, "wk": wk_dev, "wv": wv_dev,
            "wo": wo_dev, "w1": w1_dev, "w2": w2_dev,
            "b1": b1_dev, "b2": b2_dev,
            "g1r": g1_dev, "be1r": be1_dev, "g2r": g2_dev, "be2r": be2_dev,
        })
        scatter.append((bp_t, b0, np_t))
    return in_maps, scatter


def get_program():
    if "nc" not in _CACHE:
        _CACHE["nc"] = _build_program()
    return _CACHE["nc"]


def kernel(**inputs):
    in_maps, scatter = _prep_inputs(inputs)
    nc = get_program()

    from concourse.bass_utils import run_bass_kernel_spmd
    res = run_bass_kernel_spmd(nc, in_maps, list(range(NCORES)))

    Y = np.zeros((B, A, N, H), dtype=np.float32)
    for c in range(NCORES):
        bp_c, b0, np_c = scatter[c]
        Y[bp_c, b0, np_c, :] = np.asarray(res.results[c]["out"], dtype=np.float32)
    return Y


if __name__ == "__main__":
    import reference
    inputs = {k: np.asarray(v) for k, v in reference.setup_inputs().items()}
    out = kernel(**inputs)
    print("kernel output:", out.shape, out.dtype)
